# revision 7
# baseline (speedup 1.0000x reference)
import sys

if "/opt/trn_rl_repo" not in sys.path:
    sys.path.insert(0, "/opt/trn_rl_repo")

import numpy as np
import ml_dtypes

B, S, F, H, PRED = 128, 512, 128, 512, 128
NCORES = 8
BL = B // NCORES            # 16 batch rows per core
T = S + PRED - 1            # 639 total steps
G = 4 * H                   # 2048 gate rows
NB = G // 512               # 4 psum banks per layer
KH = H // 128               # 4 hidden chunks of 128

PH1_UNROLL = 4
PH2_UNROLL = 4

_cache = {}


def _build_bass():
    import concourse.bass as bass
    import concourse.tile as tile
    from concourse import mybir
    from concourse.vector_clock import ScopedClock

    # walrus in this environment accepts at most ONE sync-wait command per
    # instruction; split the Tile tail-drain waits across several drains.
    def _drain_and_barrier(self, tick_clock, wait_clock):
        drain_inst = self.nc.sync.drain()
        wait_clock.add_sem_waits(
            drain_inst.ins, ScopedClock({None: tick_clock.global_clock})
        )
        si = drain_inst.ins.sync_info
        if si is not None and si.on_wait and len(si.on_wait) > 1:
            waits = list(si.on_wait)
            si.on_wait = waits[:1]
            for w in waits[1:]:
                extra = self.nc.sync.drain()
                esi = extra.ins.sync_info
                if esi is None:
                    extra.ins.sync_info = mybir.SyncInfo(on_wait=[w], on_update=[])
                else:
                    esi.on_wait = [w]
        self.nc.all_engine_barrier()
        assert self.sems is not None
        popped = self.nc._tile_sem_poison_stack.pop()
        assert popped is self._sem_poison
        self.nc.clear_and_free_semaphores(list(self.sems.allocated().values()))
        self.nc.all_engine_barrier()

    tile.TileContext._drain_and_barrier = _drain_and_barrier

    def _split_multi_waits(nc):
        """walrus here rejects >1 sync-wait per instruction: hoist extra
        waits onto same-engine NoOps inserted just before the instruction."""
        n_split = 0
        for f in nc.m.functions:
            for blk in f.blocks:
                insts = list(blk.instructions)
                out = []
                changed = False
                for inst in insts:
                    si = inst.sync_info
                    if si is not None and si.on_wait and len(si.on_wait) > 1:
                        waits = list(si.on_wait)
                        for j, w in enumerate(waits[:-1]):
                            nop = mybir.InstNoOp(
                                name=f"{inst.name}-sw{j}", ins=[], outs=[]
                            )
                            nop.engine = inst.engine
                            nop.sync_info = mybir.SyncInfo(
                                on_wait=[w], on_update=[]
                            )
                            out.append(nop)
                            n_split += 1
                        si.on_wait = [waits[-1]]
                        changed = True
                    out.append(inst)
                if changed:
                    blk.instructions = out
        return n_split

    dt = mybir.dt
    BF = dt.bfloat16
    FP = dt.float32
    AF = mybir.ActivationFunctionType
    ds, ts = bass.ds, bass.ts

    nc = bass.Bass("TRN2", target_bir_lowering=False, debug=False)

    d_xT = nc.dram_tensor("xT", [128, S * BL], BF, kind="ExternalInput").ap()
    d_encT = nc.dram_tensor("encT", [128, 128], BF, kind="ExternalInput").ap()
    d_encb = nc.dram_tensor("encb", [128, 1], FP, kind="ExternalInput").ap()
    d_wih0 = nc.dram_tensor("wih0T", [128, G], BF, kind="ExternalInput").ap()
    d_whh0 = nc.dram_tensor("whh0T", [KH, 128, G], BF, kind="ExternalInput").ap()
    d_wih1 = nc.dram_tensor("wih1T", [KH, 128, G], BF, kind="ExternalInput").ap()
    d_whh1 = nc.dram_tensor("whh1T", [KH, 128, G], BF, kind="ExternalInput").ap()
    d_b0 = nc.dram_tensor("b0row", [1, G], BF, kind="ExternalInput").ap()
    d_b1 = nc.dram_tensor("b1row", [1, G], BF, kind="ExternalInput").ap()
    d_decT = nc.dram_tensor("decT", [KH, 128, 128], BF, kind="ExternalInput").ap()
    d_decb = nc.dram_tensor("decb", [128, 1], FP, kind="ExternalInput").ap()
    d_ones = nc.dram_tensor("ones", [1, BL], BF, kind="ExternalInput").ap()
    d_iden = nc.dram_tensor("iden", [128, 128], BF, kind="ExternalInput").ap()
    d_out = nc.dram_tensor("out", [BL, PRED * 128], FP, kind="ExternalOutput").ap()

    from contextlib import ExitStack

    with tile.TileContext(nc) as tc, ExitStack() as stk:
        cst = stk.enter_context(tc.tile_pool(name="cst", bufs=1))
        pg = stk.enter_context(tc.tile_pool(name="pg", bufs=6, space="PSUM"))
        pm = stk.enter_context(tc.tile_pool(name="pm", bufs=2, space="PSUM"))
        gt = stk.enter_context(tc.tile_pool(name="gt", bufs=2))
        hb = stk.enter_context(tc.tile_pool(name="hb", bufs=2))
        ms = stk.enter_context(tc.tile_pool(name="ms", bufs=3))
        ef = stk.enter_context(tc.tile_pool(name="ef", bufs=2))

        def load(dram_ap, shape, dtyp):
            t = cst.tile(shape, dtyp, tag=f"w{id(dram_ap)}")
            nc.gpsimd.dma_start(t[:], dram_ap)
            return t

        t_xT = cst.tile([128, S * BL], BF, tag="xT")
        nc.gpsimd.dma_start(t_xT[:], d_xT[:])
        t_encT = load(d_encT[:], [128, 128], BF)
        t_encb = load(d_encb[:], [128, 1], FP)
        t_wih0 = load(d_wih0[:], [128, G], BF)
        t_b0 = load(d_b0[:], [1, G], BF)
        t_b1 = load(d_b1[:], [1, G], BF)
        t_decb = load(d_decb[:], [128, 1], FP)
        t_ones = load(d_ones[:], [1, BL], BF)
        t_iden = load(d_iden[:], [128, 128], BF)

        t_whh0 = cst.tile([128, KH * G], BF, tag="whh0")
        t_wih1 = cst.tile([128, KH * G], BF, tag="wih1")
        t_whh1 = cst.tile([128, KH * G], BF, tag="whh1")
        t_decT = cst.tile([128, KH * 128], BF, tag="decT")
        for k in range(KH):
            nc.gpsimd.dma_start(t_whh0[:, k * G:(k + 1) * G], d_whh0[k])
            nc.gpsimd.dma_start(t_wih1[:, k * G:(k + 1) * G], d_wih1[k])
            nc.gpsimd.dma_start(t_whh1[:, k * G:(k + 1) * G], d_whh1[k])
            nc.gpsimd.dma_start(t_decT[:, k * 128:(k + 1) * 128], d_decT[k])

        # persistent state
        t_eT = cst.tile([128, S * BL], BF, tag="eT")
        t_h0T = cst.tile([128, KH * BL], BF, tag="h0T")
        t_h1T = cst.tile([128, KH * BL], BF, tag="h1T")
        t_c0 = cst.tile([BL, H], FP, tag="c0")
        t_c1 = cst.tile([BL, H], FP, tag="c1")
        t_eCur = cst.tile([128, BL], BF, tag="eCur")
        t_osb = cst.tile([BL, PRED * 128], FP, tag="osb")

        nc.gpsimd.memset(t_h0T[:], 0)
        nc.gpsimd.memset(t_h1T[:], 0)
        nc.gpsimd.memset(t_c0[:], 0)
        nc.gpsimd.memset(t_c1[:], 0)

        # ---- encoder precompute: eT[:, t*BL+b] = enc_w @ x_t,b + enc_b ----
        for s_ in range(S * BL // 512):
            ep = pg.tile([128, 512], FP, tag="gp")
            nc.tensor.matmul(
                ep[:], t_encT[:], t_xT[:, 512 * s_:512 * (s_ + 1)],
                start=True, stop=True,
            )
            nc.scalar.activation(
                t_eT[:, 512 * s_:512 * (s_ + 1)], ep[:], AF.Identity,
                bias=t_encb[:],
            )

        SIG = AF.Sigmoid
        TANH = AF.Tanh

        def lstm_layer(inpT_list, wihT, hT, whhT, brow, c, outT):
            """One LSTM cell update (batch-major gates, [i,f,o,g] banks)."""
            gps = []
            for n in range(NB):
                gp = pg.tile([BL, 512], FP, tag="gp")
                sl = slice(n * 512, (n + 1) * 512)
                nc.tensor.matmul(gp[:], t_ones[:], brow[:, sl], start=True, stop=False)
                for (lhs, w) in inpT_list:
                    nc.tensor.matmul(gp[:], lhs, w[:, sl], start=False, stop=False)
                for k in range(KH):
                    nc.tensor.matmul(
                        gp[:], hT[:, k * BL:(k + 1) * BL],
                        whhT[:, k * G + n * 512: k * G + (n + 1) * 512],
                        start=False, stop=(k == KH - 1),
                    )
                gps.append(gp)
            s_i = gt.tile([BL, 512], FP, tag="s_i")
            s_f = gt.tile([BL, 512], FP, tag="s_f")
            s_o = gt.tile([BL, 512], FP, tag="s_o")
            tg = gt.tile([BL, 512], FP, tag="tg")
            nc.scalar.activation(s_i[:], gps[0][:], SIG)
            nc.scalar.activation(s_f[:], gps[1][:], SIG)
            nc.scalar.activation(s_o[:], gps[2][:], SIG)
            nc.scalar.activation(tg[:], gps[3][:], TANH)
            t1 = gt.tile([BL, 512], FP, tag="t1")
            t2 = gt.tile([BL, 512], FP, tag="t2")
            nc.vector.tensor_mul(t1[:], s_i[:], tg[:])
            nc.vector.tensor_mul(t2[:], s_f[:], c[:])
            nc.vector.tensor_add(c[:], t2[:], t1[:])
            th = gt.tile([BL, 512], FP, tag="th")
            nc.scalar.activation(th[:], c[:], TANH)
            hbf = hb.tile([BL, H], BF, tag="hbf")
            nc.vector.tensor_mul(hbf[:], s_o[:], th[:])
            for k in range(KH):
                tp = pm.tile([128, BL], BF, tag="pm")
                nc.tensor.transpose(
                    tp[:], hbf[:, 128 * k:128 * (k + 1)], t_iden[0:BL, 0:BL]
                )
                nc.vector.tensor_copy(outT[:, k * BL:(k + 1) * BL], tp[:])

        def h_chunks(hT):
            return [(hT[:, k * BL:(k + 1) * BL],) for k in range(KH)]

        def step_core(inp_lhs):
            # layer 0: ih from encoder output, hh recurrent
            lstm_layer(
                [(inp_lhs, t_wih0)], None, t_h0T, t_whh0, t_b0, t_c0, t_h0T
            )
            # layer 1: ih from h0 (4 chunks), hh recurrent
            ih1 = [
                (t_h0T[:, k * BL:(k + 1) * BL],
                 t_wih1[:, k * G:(k + 1) * G])
                for k in range(KH)
            ]
            lstm_layer(ih1, None, t_h1T, t_whh1, t_b1, t_c1, t_h1T)

        def dec_store_enc(out_col):
            # decoder -> outT (gate-major [i, b]), with bias; feeds back + store
            op = pm.tile([128, BL], FP, tag="pm")
            for k in range(KH):
                nc.tensor.matmul(
                    op[:], t_decT[:, k * 128:(k + 1) * 128],
                    t_h1T[:, k * BL:(k + 1) * BL],
                    start=(k == 0), stop=(k == KH - 1),
                )
            outTb = ms.tile([128, BL], BF, tag="outTb")
            nc.scalar.activation(outTb[:], op[:], AF.Identity, bias=t_decb[:])
            on = pm.tile([BL, 128], BF, tag="pm")
            nc.tensor.transpose(on[:], outTb[:], t_iden[:])
            nc.vector.tensor_copy(t_osb[:, out_col], on[:])
            # encoder on fed-back output -> eCur
            ep2 = pm.tile([128, BL], FP, tag="pm")
            nc.tensor.matmul(ep2[:], t_encT[:], outTb[:], start=True, stop=True)
            nc.scalar.activation(t_eCur[:], ep2[:], AF.Identity, bias=t_encb[:])

        # ---- phase 1: teacher forced, t = 0..510 (no decoder output) ----
        # matmul lhsT cannot take register offsets; stage eT slice through a
        # fixed-address tile with a cheap DVE copy.
        def ph1_body(t):
            eFix = ef.tile([128, BL], BF, tag="ef")
            nc.vector.tensor_copy(eFix[:], t_eT[:, ts(t, BL)])
            step_core(eFix[:])

        tc.For_i_unrolled(0, S - 1, 1, ph1_body, max_unroll=PH1_UNROLL)

        # ---- step t = 511: teacher forced input, first decoded output ----
        step_core(t_eT[:, (S - 1) * BL:S * BL])
        dec_store_enc(slice(0, 128))

        # ---- phase 2: autoregressive, t2 = 1..127 (steps 512..638) ----
        def ph2_body(t2):
            step_core(t_eCur[:])
            dec_store_enc(ds(t2 * 128, 128))

        tc.For_i_unrolled(1, PRED, 1, ph2_body, max_unroll=PH2_UNROLL)

        nc.gpsimd.dma_start(d_out[:], t_osb[:])

    _split_multi_waits(nc)
    return nc


def _prep_shared(enc_w, enc_b, dec_w, dec_b, wih0, whh0, bih0, bhh0,
                 wih1, whh1, bih1, bhh1):
    bf = ml_dtypes.bfloat16
    perm = np.r_[0:H, H:2 * H, 3 * H:4 * H, 2 * H:3 * H]  # [i,f,g,o]->[i,f,o,g]
    shared = {
        "encT": np.ascontiguousarray(enc_w.T).astype(bf),
        "encb": np.ascontiguousarray(enc_b.reshape(128, 1)).astype(np.float32),
        "wih0T": np.ascontiguousarray(wih0[perm].T).astype(bf),
        "whh0T": np.ascontiguousarray(
            whh0[perm].T.reshape(KH, 128, G)).astype(bf),
        "wih1T": np.ascontiguousarray(
            wih1[perm].T.reshape(KH, 128, G)).astype(bf),
        "whh1T": np.ascontiguousarray(
            whh1[perm].T.reshape(KH, 128, G)).astype(bf),
        "b0row": np.ascontiguousarray(
            (bih0 + bhh0)[perm].reshape(1, G)).astype(bf),
        "b1row": np.ascontiguousarray(
            (bih1 + bhh1)[perm].reshape(1, G)).astype(bf),
        "decT": np.ascontiguousarray(
            dec_w.T.reshape(KH, 128, 128)).astype(bf),
        "decb": np.ascontiguousarray(dec_b.reshape(128, 1)).astype(np.float32),
        "ones": np.ones((1, BL), bf),
        "iden": np.eye(128, dtype=np.float32).astype(bf),
    }
    return shared


def run(x, enc_w, enc_b, dec_w, dec_b, wih0, whh0, bih0, bhh0,
        wih1, whh1, bih1, bhh1, pred_len, trace=False):
    from concourse.bass_utils import run_bass_kernel_spmd

    assert int(pred_len) == PRED
    x = np.asarray(x, np.float32)
    if "nc" not in _cache:
        _cache["nc"] = _build_bass()
    nc = _cache["nc"]

    shared = _prep_shared(
        np.asarray(enc_w, np.float32), np.asarray(enc_b, np.float32),
        np.asarray(dec_w, np.float32), np.asarray(dec_b, np.float32),
        np.asarray(wih0, np.float32), np.asarray(whh0, np.float32),
        np.asarray(bih0, np.float32), np.asarray(bhh0, np.float32),
        np.asarray(wih1, np.float32), np.asarray(whh1, np.float32),
        np.asarray(bih1, np.float32), np.asarray(bhh1, np.float32))

    bf = ml_dtypes.bfloat16
    in_maps = []
    for c in range(NCORES):
        xs = x[c * BL:(c + 1) * BL]                       # [BL, S, F]
        xT = np.ascontiguousarray(xs.transpose(2, 1, 0))  # [F, S, BL]
        m = dict(shared)
        m["xT"] = xT.reshape(128, S * BL).astype(bf)
        in_maps.append(m)

    res = run_bass_kernel_spmd(
        nc, in_maps, core_ids=list(range(NCORES)), trace=trace
    )
    outs = [res.results[c]["out"].reshape(BL, PRED, 128) for c in range(NCORES)]
    full = np.concatenate(outs, axis=0).astype(np.float32)
    return full, res


def kernel(**inputs):
    out, _ = run(**inputs)
    return out


# revision 16
# speedup vs baseline: 254.4466x; 254.4466x over previous
import sys

if "/opt/trn_rl_repo" not in sys.path:
    sys.path.insert(0, "/opt/trn_rl_repo")

import numpy as np
import ml_dtypes

B, S, F, H, PRED = 128, 512, 128, 512, 128
NCORES = 8
BL = B // NCORES            # 16 batch rows per core
T = S + PRED - 1            # 639 total steps
G = 4 * H                   # 2048 gate rows
NB = G // 512               # 4 psum banks per layer
KH = H // 128               # 4 hidden chunks of 128

PH1_UNROLL = 8
PH2_UNROLL = 4

_cache = {}


def _build_bass():
    import concourse.bass as bass
    import concourse.tile as tile
    from concourse import mybir
    from concourse.vector_clock import ScopedClock

    # walrus in this environment accepts at most ONE sync-wait command per
    # instruction; split the Tile tail-drain waits across several drains.
    def _drain_and_barrier(self, tick_clock, wait_clock):
        drain_inst = self.nc.sync.drain()
        wait_clock.add_sem_waits(
            drain_inst.ins, ScopedClock({None: tick_clock.global_clock})
        )
        si = drain_inst.ins.sync_info
        if si is not None and si.on_wait and len(si.on_wait) > 1:
            waits = list(si.on_wait)
            si.on_wait = waits[:1]
            for w in waits[1:]:
                extra = self.nc.sync.drain()
                esi = extra.ins.sync_info
                if esi is None:
                    extra.ins.sync_info = mybir.SyncInfo(on_wait=[w], on_update=[])
                else:
                    esi.on_wait = [w]
        self.nc.all_engine_barrier()
        assert self.sems is not None
        popped = self.nc._tile_sem_poison_stack.pop()
        assert popped is self._sem_poison
        self.nc.clear_and_free_semaphores(list(self.sems.allocated().values()))
        self.nc.all_engine_barrier()

    tile.TileContext._drain_and_barrier = _drain_and_barrier

    def _split_multi_waits(nc):
        """walrus here rejects >1 sync-wait per instruction: hoist extra
        waits onto same-engine NoOps inserted just before the instruction."""
        n_split = 0
        for f in nc.m.functions:
            for blk in f.blocks:
                insts = list(blk.instructions)
                out = []
                changed = False
                for inst in insts:
                    si = inst.sync_info
                    if si is not None and si.on_wait and len(si.on_wait) > 1:
                        waits = list(si.on_wait)
                        for j, w in enumerate(waits[:-1]):
                            nop = mybir.InstNoOp(
                                name=f"{inst.name}-sw{j}", ins=[], outs=[]
                            )
                            nop.engine = inst.engine
                            nop.sync_info = mybir.SyncInfo(
                                on_wait=[w], on_update=[]
                            )
                            out.append(nop)
                            n_split += 1
                        si.on_wait = [waits[-1]]
                        changed = True
                    out.append(inst)
                if changed:
                    blk.instructions = out
        return n_split

    dt = mybir.dt
    BF = dt.bfloat16
    FP = dt.float32
    AF = mybir.ActivationFunctionType
    ds, ts = bass.ds, bass.ts

    nc = bass.Bass("TRN2", target_bir_lowering=False, debug=False)

    d_xT = nc.dram_tensor("xT", [128, S * BL], BF, kind="ExternalInput").ap()
    d_encT = nc.dram_tensor("encT", [128, 128], BF, kind="ExternalInput").ap()
    d_encb = nc.dram_tensor("encb", [128, 1], FP, kind="ExternalInput").ap()
    d_wih0 = nc.dram_tensor("wih0T", [128, G], BF, kind="ExternalInput").ap()
    d_whh0 = nc.dram_tensor("whh0T", [KH, 128, G], BF, kind="ExternalInput").ap()
    d_wih1 = nc.dram_tensor("wih1T", [KH, 128, G], BF, kind="ExternalInput").ap()
    d_whh1 = nc.dram_tensor("whh1T", [KH, 128, G], BF, kind="ExternalInput").ap()
    d_b0 = nc.dram_tensor("b0row", [1, G], BF, kind="ExternalInput").ap()
    d_b1 = nc.dram_tensor("b1row", [1, G], BF, kind="ExternalInput").ap()
    d_decT = nc.dram_tensor("decT", [KH, 128, 128], BF, kind="ExternalInput").ap()
    d_decb = nc.dram_tensor("decb", [128, 1], FP, kind="ExternalInput").ap()
    d_ones = nc.dram_tensor("ones", [1, BL], BF, kind="ExternalInput").ap()
    d_iden = nc.dram_tensor("iden", [128, 128], BF, kind="ExternalInput").ap()
    d_out = nc.dram_tensor("out", [BL, PRED * 128], FP, kind="ExternalOutput").ap()

    from contextlib import ExitStack

    with tile.TileContext(nc) as tc, ExitStack() as stk:
        cst = stk.enter_context(tc.tile_pool(name="cst", bufs=1))
        pg = stk.enter_context(tc.tile_pool(name="pg", bufs=6, space="PSUM"))
        pm = stk.enter_context(tc.tile_pool(name="pm", bufs=2, space="PSUM"))
        gt = stk.enter_context(tc.tile_pool(name="gt", bufs=2))
        hb = stk.enter_context(tc.tile_pool(name="hb", bufs=2))
        ms = stk.enter_context(tc.tile_pool(name="ms", bufs=3))
        ef = stk.enter_context(tc.tile_pool(name="ef", bufs=2))

        def load(dram_ap, shape, dtyp):
            t = cst.tile(shape, dtyp, tag=f"w{id(dram_ap)}")
            nc.gpsimd.dma_start(t[:], dram_ap)
            return t

        t_xT = cst.tile([128, S * BL], BF, tag="xT")
        nc.gpsimd.dma_start(t_xT[:], d_xT[:])
        t_encT = load(d_encT[:], [128, 128], BF)
        t_encb = load(d_encb[:], [128, 1], FP)
        t_wih0 = load(d_wih0[:], [128, G], BF)
        t_b0 = load(d_b0[:], [1, G], BF)
        t_b1 = load(d_b1[:], [1, G], BF)
        t_decb = load(d_decb[:], [128, 1], FP)
        t_ones = load(d_ones[:], [1, BL], BF)
        t_iden = load(d_iden[:], [128, 128], BF)

        t_whh0 = cst.tile([128, KH * G], BF, tag="whh0")
        t_wih1 = cst.tile([128, KH * G], BF, tag="wih1")
        t_whh1 = cst.tile([128, KH * G], BF, tag="whh1")
        t_decT = cst.tile([128, KH * 128], BF, tag="decT")
        for k in range(KH):
            nc.gpsimd.dma_start(t_whh0[:, k * G:(k + 1) * G], d_whh0[k])
            nc.gpsimd.dma_start(t_wih1[:, k * G:(k + 1) * G], d_wih1[k])
            nc.gpsimd.dma_start(t_whh1[:, k * G:(k + 1) * G], d_whh1[k])
            nc.gpsimd.dma_start(t_decT[:, k * 128:(k + 1) * 128], d_decT[k])

        # persistent state
        t_eT = cst.tile([128, S * BL], BF, tag="eT")
        t_h0T = cst.tile([128, KH * BL], BF, tag="h0T")
        t_h1T = cst.tile([128, KH * BL], BF, tag="h1T")
        t_c0 = cst.tile([BL, H], FP, tag="c0")
        t_c1 = cst.tile([BL, H], FP, tag="c1")
        t_eCur = cst.tile([128, BL], BF, tag="eCur")
        t_osb = cst.tile([BL, PRED * 128], FP, tag="osb")

        nc.gpsimd.memset(t_h0T[:], 0)
        nc.gpsimd.memset(t_h1T[:], 0)
        nc.gpsimd.memset(t_c0[:], 0)
        nc.gpsimd.memset(t_c1[:], 0)

        # ---- encoder precompute: eT[:, t*BL+b] = enc_w @ x_t,b + enc_b ----
        for s_ in range(S * BL // 512):
            ep = pg.tile([128, 512], FP, tag="gp")
            nc.tensor.matmul(
                ep[:], t_encT[:], t_xT[:, 512 * s_:512 * (s_ + 1)],
                start=True, stop=True,
            )
            nc.scalar.activation(
                t_eT[:, 512 * s_:512 * (s_ + 1)], ep[:], AF.Identity,
                bias=t_encb[:],
            )

        SIG = AF.Sigmoid
        TANH = AF.Tanh

        def gate_mms_a(hT, whhT, brow):
            """bias + recurrent (hh) matmuls — independent of this step's
            input; emitted early so the PE stays busy during gate latency."""
            gps = []
            for n in range(NB):
                gp = pg.tile([BL, 512], FP, tag="gp")
                sl = slice(n * 512, (n + 1) * 512)
                nc.tensor.matmul(gp[:], t_ones[:], brow[:, sl], start=True, stop=False)
                for k in range(KH):
                    nc.tensor.matmul(
                        gp[:], hT[:, k * BL:(k + 1) * BL],
                        whhT[:, k * G + n * 512: k * G + (n + 1) * 512],
                        start=False, stop=False,
                    )
                gps.append(gp)
            return gps

        def gate_mms_b(gps, inpT_list):
            """ih matmuls (depend on the freshly computed input/h0)."""
            for n in range(NB):
                sl = slice(n * 512, (n + 1) * 512)
                for j, (lhs, w) in enumerate(inpT_list):
                    nc.tensor.matmul(
                        gps[n][:], lhs, w[:, sl],
                        start=False, stop=(j == len(inpT_list) - 1),
                    )

        def gate_tail(gps, c, outT):
            """sigmoid/tanh, cell update, h -> bf16, transpose into outT."""
            s_i = gt.tile([BL, 512], FP, tag="s_i")
            s_f = gt.tile([BL, 512], FP, tag="s_f")
            s_o = gt.tile([BL, 512], FP, tag="s_o")
            tg = gt.tile([BL, 512], FP, tag="tg")
            nc.scalar.activation(s_i[:], gps[0][:], SIG)
            nc.scalar.activation(s_f[:], gps[1][:], SIG)
            nc.scalar.activation(s_o[:], gps[2][:], SIG)
            nc.scalar.activation(tg[:], gps[3][:], TANH)
            t1 = gt.tile([BL, 512], FP, tag="t1")
            t2 = gt.tile([BL, 512], FP, tag="t2")
            nc.vector.tensor_mul(t1[:], s_i[:], tg[:])
            nc.vector.tensor_mul(t2[:], s_f[:], c[:])
            nc.vector.tensor_add(c[:], t2[:], t1[:])
            th = gt.tile([BL, 512], FP, tag="th")
            nc.scalar.activation(th[:], c[:], TANH)
            hbf = hb.tile([BL, H], BF, tag="hbf")
            nc.vector.tensor_mul(hbf[:], s_o[:], th[:])
            for k in range(KH):
                tp = pm.tile([128, BL], BF, tag="pm")
                nc.tensor.transpose(
                    tp[:], hbf[:, 128 * k:128 * (k + 1)], t_iden[0:BL, 0:BL]
                )
                nc.vector.tensor_copy(outT[:, k * BL:(k + 1) * BL], tp[:])

        def step_core(inp_lhs):
            # L0 matmuls: bias+hh then ih (all inputs ready at step start)
            gps0 = gate_mms_a(t_h0T, t_whh0, t_b0)
            gate_mms_b(gps0, [(inp_lhs, t_wih0)])
            # L1 bias+hh: depends only on h1 from last step — keeps PE busy
            # while L0's gate tail (ACT/DVE) runs
            gps1 = gate_mms_a(t_h1T, t_whh1, t_b1)
            # L0 gate tail produces h0' (+ its transpose into t_h0T)
            gate_tail(gps0, t_c0, t_h0T)
            # L1 ih matmuls consume fresh h0'
            ih1 = [
                (t_h0T[:, k * BL:(k + 1) * BL],
                 t_wih1[:, k * G:(k + 1) * G])
                for k in range(KH)
            ]
            gate_mms_b(gps1, ih1)
            gate_tail(gps1, t_c1, t_h1T)

        def dec_store_enc(out_col):
            # decoder -> outT (gate-major [i, b]), with bias; feeds back + store
            op = pm.tile([128, BL], FP, tag="pm")
            for k in range(KH):
                nc.tensor.matmul(
                    op[:], t_decT[:, k * 128:(k + 1) * 128],
                    t_h1T[:, k * BL:(k + 1) * BL],
                    start=(k == 0), stop=(k == KH - 1),
                )
            outTb = ms.tile([128, BL], BF, tag="outTb")
            nc.scalar.activation(outTb[:], op[:], AF.Identity, bias=t_decb[:])
            on = pm.tile([BL, 128], BF, tag="pm")
            nc.tensor.transpose(on[:], outTb[:], t_iden[:])
            nc.scalar.copy(t_osb[:, out_col], on[:])
            # encoder on fed-back output -> eCur
            ep2 = pm.tile([128, BL], FP, tag="pm")
            nc.tensor.matmul(ep2[:], t_encT[:], outTb[:], start=True, stop=True)
            nc.scalar.activation(t_eCur[:], ep2[:], AF.Identity, bias=t_encb[:])

        # ---- phase 1: teacher forced, t = 0..510 (no decoder output) ----
        # matmul lhsT cannot take register offsets; stage eT slice through a
        # fixed-address tile with a cheap DVE copy.
        def ph1_body(t):
            eFix = ef.tile([128, BL], BF, tag="ef")
            nc.gpsimd.tensor_copy(eFix[:], t_eT[:, ts(t, BL)])
            step_core(eFix[:])

        tc.For_i_unrolled(0, S - 1, 1, ph1_body, max_unroll=PH1_UNROLL)

        # ---- step t = 511: teacher forced input, first decoded output ----
        step_core(t_eT[:, (S - 1) * BL:S * BL])
        dec_store_enc(slice(0, 128))

        # ---- phase 2: autoregressive, t2 = 1..127 (steps 512..638) ----
        def ph2_body(t2):
            step_core(t_eCur[:])
            dec_store_enc(ds(t2 * 128, 128))

        tc.For_i_unrolled(1, PRED, 1, ph2_body, max_unroll=PH2_UNROLL)

        nc.gpsimd.dma_start(d_out[:], t_osb[:])

    _split_multi_waits(nc)
    return nc


def _prep_shared(enc_w, enc_b, dec_w, dec_b, wih0, whh0, bih0, bhh0,
                 wih1, whh1, bih1, bhh1):
    bf = ml_dtypes.bfloat16
    perm = np.r_[0:H, H:2 * H, 3 * H:4 * H, 2 * H:3 * H]  # [i,f,g,o]->[i,f,o,g]
    shared = {
        "encT": np.ascontiguousarray(enc_w.T).astype(bf),
        "encb": np.ascontiguousarray(enc_b.reshape(128, 1)).astype(np.float32),
        "wih0T": np.ascontiguousarray(wih0[perm].T).astype(bf),
        "whh0T": np.ascontiguousarray(
            whh0[perm].T.reshape(KH, 128, G)).astype(bf),
        "wih1T": np.ascontiguousarray(
            wih1[perm].T.reshape(KH, 128, G)).astype(bf),
        "whh1T": np.ascontiguousarray(
            whh1[perm].T.reshape(KH, 128, G)).astype(bf),
        "b0row": np.ascontiguousarray(
            (bih0 + bhh0)[perm].reshape(1, G)).astype(bf),
        "b1row": np.ascontiguousarray(
            (bih1 + bhh1)[perm].reshape(1, G)).astype(bf),
        "decT": np.ascontiguousarray(
            dec_w.T.reshape(KH, 128, 128)).astype(bf),
        "decb": np.ascontiguousarray(dec_b.reshape(128, 1)).astype(np.float32),
        "ones": np.ones((1, BL), bf),
        "iden": np.eye(128, dtype=np.float32).astype(bf),
    }
    return shared


def run(x, enc_w, enc_b, dec_w, dec_b, wih0, whh0, bih0, bhh0,
        wih1, whh1, bih1, bhh1, pred_len, trace=False):
    from concourse.bass_utils import run_bass_kernel_spmd

    assert int(pred_len) == PRED
    x = np.asarray(x, np.float32)
    if "nc" not in _cache:
        _cache["nc"] = _build_bass()
    nc = _cache["nc"]

    shared = _prep_shared(
        np.asarray(enc_w, np.float32), np.asarray(enc_b, np.float32),
        np.asarray(dec_w, np.float32), np.asarray(dec_b, np.float32),
        np.asarray(wih0, np.float32), np.asarray(whh0, np.float32),
        np.asarray(bih0, np.float32), np.asarray(bhh0, np.float32),
        np.asarray(wih1, np.float32), np.asarray(whh1, np.float32),
        np.asarray(bih1, np.float32), np.asarray(bhh1, np.float32))

    bf = ml_dtypes.bfloat16
    in_maps = []
    for c in range(NCORES):
        xs = x[c * BL:(c + 1) * BL]                       # [BL, S, F]
        xT = np.ascontiguousarray(xs.transpose(2, 1, 0))  # [F, S, BL]
        m = dict(shared)
        m["xT"] = xT.reshape(128, S * BL).astype(bf)
        in_maps.append(m)

    res = run_bass_kernel_spmd(
        nc, in_maps, core_ids=list(range(NCORES)), trace=trace
    )
    outs = [res.results[c]["out"].reshape(BL, PRED, 128) for c in range(NCORES)]
    full = np.concatenate(outs, axis=0).astype(np.float32)
    return full, res


def kernel(**inputs):
    out, _ = run(**inputs)
    return out
